# revision 1
# baseline (speedup 1.0000x reference)
"""GAT (2-layer graph attention network) on 8 Trainium2 NeuronCores.

Strategy (per spec sharding hint): shard the node dim N=4096 across 8 cores
(512 rows each). Each core computes its [512, 4096] slice of each attention
matrix; row-wise softmax is local. h (layer-1 features) and h_out (layer-2
input) are all-gathered across cores via AllGather collectives.

Key kernel structure (per core), all in "transposed" layout [j-partition,
i-free] so the att @ h contraction lands on the PE partition dim:

  e[i,j] = leakyrelu(s_src[i] + s_dst[j]) is rank-1 before the nonlinearity:
  s_src/s_dst are tiny per-node scalars, so no QK matmul is needed.
  The mask is folded additively pre-lrelu: t = s_src + s_dst + (adj-1)*1500
  => exp(lrelu(t)) == adj * exp(lrelu(e)) exactly in f32 (underflow to 0).

  Per (head, j-block) tile [128j, 512i]:
    STT-1 (DVE scalar_tensor_tensor): t = (src_bcast + s_dst[jb]) + mask_bias
    lrelu: DVE STT (t*0.2 max t) for 3/4 tiles, ACT Prelu for 1/4 (balance)
    ACT Exp (batched over 4 jb)  -> p tile (bf16)
    PE matmul: psum[65, 512] += [h[jb] | ones].T @ p   (ones row = softmax
    denominator, accumulated over all 32 j-blocks)

  Then per head: normalize by the denominator row, ELU, assemble x_catT;
  h_out = x_catT @ W_out; AllGather h_out; layer-2 attention (same scheme,
  row-form output) and log_softmax.
"""
import sys
import time

sys.path.insert(0, "/opt/trn_rl_repo")

import numpy as np
import ml_dtypes

import concourse.bass as bass
import concourse.bacc as bacc
import concourse.tile as tile
from concourse import mybir
from concourse.bass_utils import run_bass_kernel_spmd
from concourse.masks import make_identity

dt = mybir.dt
BF = ml_dtypes.bfloat16

N, NFEAT, NHID, NHEAD, NCLASS = 4096, 1024, 64, 8, 32
NCORES = 8
R = N // NCORES          # 512 rows per core
NJB = N // 128           # 32 j-blocks
KCH = NFEAT // 128       # 8 full K chunks for x@W (+1 for bias row)
MASK_BIG = 1500.0
ALPHA = 0.2

_cached = {}


def _build_program():
    nc = bacc.Bacc("TRN2", target_bir_lowering=False, debug=False,
                   enable_asserts=False, num_devices=NCORES)

    xT = nc.dram_tensor("xT", [NFEAT + 1, R], dt.bfloat16, kind="ExternalInput").ap()
    wh = nc.dram_tensor("wh", [NHEAD, NFEAT + 1, NHID], dt.bfloat16, kind="ExternalInput").ap()
    adjT = nc.dram_tensor("adjT", [N, R], dt.bfloat16, kind="ExternalInput").ap()
    aT = nc.dram_tensor("aT", [NHEAD, NHID, 2], dt.bfloat16, kind="ExternalInput").ap()
    wo = nc.dram_tensor("wo", [NHEAD * NHID + 1, NCLASS], dt.bfloat16, kind="ExternalInput").ap()
    ao = nc.dram_tensor("ao", [NCLASS, 2], dt.float32, kind="ExternalInput").ap()
    out = nc.dram_tensor("out", [R, NCLASS], dt.float32, kind="ExternalOutput").ap()

    with tile.TileContext(nc, num_cores=NCORES) as tc:
        _emit(nc, tc, xT, wh, adjT, aT, wo, ao, out)
    nc.compile()
    return nc


def _emit(nc, tc, xT, wh, adjT, aT, wo, ao, out):
    from contextlib import ExitStack
    f32, bf16 = dt.float32, dt.bfloat16
    AF = mybir.ActivationFunctionType
    OP = mybir.AluOpType
    AG = "AllGather"

    cst_ctx = ExitStack()
    cst = cst_ctx.enter_context(tc.tile_pool(name="cst", bufs=1))
    dram = cst_ctx.enter_context(tc.tile_pool(name="dram", bufs=1, space="DRAM"))

    # ---- collective buffers ----
    cc_s_in = dram.tile([2 * NHEAD, R], f32)
    cc_s_out = dram.tile([NCORES, 2 * NHEAD, R], f32, addr_space="Shared")
    cc_h_in = [dram.tile([R, NHID], bf16, name=f"cc_h_in{h}") for h in range(NHEAD)]
    cc_h_out = [dram.tile([NCORES, R, NHID], bf16, addr_space="Shared",
                          name=f"cc_h_out{h}") for h in range(NHEAD)]
    cc_ho_in = dram.tile([R, NCLASS], bf16)
    cc_ho_out = dram.tile([NCORES, R, NCLASS], bf16, addr_space="Shared")
    cc_s2_in = dram.tile([2, R], f32)
    cc_s2_out = dram.tile([NCORES, 2, R], f32, addr_space="Shared")
    groups = [list(range(NCORES))]

    # ---- persistent SBUF ----
    mT = cst.tile([128, NJB, R], bf16)            # raw 0/1 mask, transposed

    h_rhs = [cst.tile([128, NJB, NHID + 1], bf16, name=f"h_rhs{h}")
             for h in range(NHEAD)]
    for h in range(NHEAD):
        nc.vector.memset(h_rhs[h][:, :, NHID:NHID + 1], 1.0)

    src_bc = [cst.tile([128, R], bf16, name=f"src_bc{h}") for h in range(NHEAD)]
    src02_bc = [cst.tile([128, R], bf16, name=f"src02_bc{h}") for h in range(NHEAD)]
    sdst = cst.tile([128, NHEAD, NJB], f32)
    sdst02 = cst.tile([128, NHEAD, NJB], f32)
    ident64 = cst.tile([64, 64], bf16)
    make_identity(nc, ident64)
    ident128 = cst.tile([128, 128], f32)
    make_identity(nc, ident128)
    ident33 = cst.tile([NCLASS + 1, NCLASS + 1], f32)
    make_identity(nc, ident33)
    ones64 = cst.tile([1, 64], f32)
    nc.vector.memset(ones64, 1.0)
    ones128 = cst.tile([1, 128], f32)
    nc.vector.memset(ones128, 1.0)
    ones_row = cst.tile([1, R], bf16)
    nc.vector.memset(ones_row, 1.0)
    xcatT = [cst.tile([128, R], bf16, name=f"xcatT{k}") for k in range(4)]
    h2_rhs = cst.tile([128, NJB, NCLASS + 1], bf16)
    nc.vector.memset(h2_rhs[:, :, NCLASS:NCLASS + 1], 1.0)
    src2_bc = cst.tile([128, R], bf16)
    src202_bc = cst.tile([128, R], bf16)
    s2dst = cst.tile([128, NJB], f32)
    s2dst02 = cst.tile([128, NJB], f32)

    # =================== Stage A: h = x @ W per head, s vectors ============
    stA = ExitStack()
    sa = stA.enter_context(tc.tile_pool(name="sa", bufs=1))
    psA = stA.enter_context(tc.tile_pool(name="psA", bufs=1, space="PSUM"))

    xT_sb = sa.tile([128, KCH + 1, R], bf16)
    nc.sync.dma_start(out=xT_sb[:, 0:KCH, :],
                      in_=xT[0:NFEAT, :].rearrange("(k p) i -> p k i", p=128))
    nc.sync.dma_start(out=xT_sb[0:1, KCH, :], in_=xT[NFEAT:NFEAT + 1, :])
    wh_sb = sa.tile([128, NHEAD, KCH + 1, NHID], bf16)
    for h in range(NHEAD):
        nc.sync.dma_start(out=wh_sb[:, h, 0:KCH, :],
                           in_=wh[h, 0:NFEAT, :].rearrange("(k p) o -> p k o", p=128))
        nc.sync.dma_start(out=wh_sb[0:1, h, KCH, :], in_=wh[h, NFEAT:NFEAT + 1, :])
    aT_sb = sa.tile([64, NHEAD, 2], bf16)
    nc.sync.dma_start(out=aT_sb, in_=aT.rearrange("h o k -> o h k"))
    nc.sync.dma_start(
        out=mT,
        in_=adjT.rearrange("(jb p) i -> p jb i", p=128))

    hT_sb = sa.tile([64, NHEAD, R], bf16)
    for h in range(NHEAD):
        ps_hT = psA.tile([64, R], f32, tag="hT", bufs=2)
        for k in range(KCH + 1):
            kp = 128 if k < KCH else 1
            nc.tensor.matmul(ps_hT, lhsT=wh_sb[0:kp, h, k, :],
                             rhs=xT_sb[0:kp, k, :],
                             start=(k == 0), stop=(k == KCH))
        nc.scalar.copy(out=hT_sb[:, h, :], in_=ps_hT)
        ps_s1 = psA.tile([2, R], f32, tag="s1", bufs=2)
        nc.tensor.matmul(ps_s1, lhsT=aT_sb[:, h, :], rhs=hT_sb[:, h, :],
                         start=True, stop=True)
        s1_sb = sa.tile([2, R], f32, tag="s1sb", bufs=2)
        nc.vector.tensor_copy(out=s1_sb, in_=ps_s1)
        nc.sync.dma_start(out=cc_s_in[2 * h:2 * h + 2, :], in_=s1_sb)
        ps_src = psA.tile([128, R], f32, tag="srcbc", bufs=2)
        nc.tensor.matmul(ps_src, lhsT=ones128, rhs=s1_sb[0:1, :],
                         start=True, stop=True)
        nc.vector.tensor_copy(out=src_bc[h], in_=ps_src)
        nc.vector.tensor_scalar(out=src02_bc[h], in0=ps_src, scalar1=ALPHA,
                                scalar2=None, op0=OP.mult)
        h_row4 = sa.tile([128, 4, 64], bf16, tag="hrow", bufs=2)
        for tb in range(4):
            ps_tr = psA.tile([128, 64], bf16, tag="tr", bufs=2)
            nc.tensor.transpose(ps_tr, hT_sb[:, h, tb * 128:(tb + 1) * 128], ident64)
            nc.vector.tensor_copy(out=h_row4[:, tb, :], in_=ps_tr)
        nc.sync.dma_start(out=cc_h_in[h].rearrange("(l p) o -> p l o", p=128),
                          in_=h_row4)

    nc.gpsimd.collective_compute(AG, mybir.AluOpType.bypass, replica_groups=groups,
                                 ins=[cc_s_in[:]], outs=[cc_s_out[:]])
    for h in range(NHEAD):
        nc.gpsimd.collective_compute(AG, mybir.AluOpType.bypass, replica_groups=groups,
                                     ins=[cc_h_in[h][:]], outs=[cc_h_out[h][:]])

    # sdst tiles first (small, unblock the attend pipeline), then h_rhs
    for h in range(NHEAD):
        for core in range(NCORES):
            nc.sync.dma_start(
                out=sdst[:, h, core * 4:(core + 1) * 4],
                in_=cc_s_out[core, 2 * h + 1, :].rearrange("(l p) -> p l", p=128))
    sdst2d = sdst.rearrange("p a b -> p (a b)")
    sdst02_2d = sdst02.rearrange("p a b -> p (a b)")
    nc.vector.tensor_scalar(out=sdst02_2d, in0=sdst2d, scalar1=ALPHA,
                            scalar2=None, op0=OP.mult)
    for h in range(NHEAD):
        for core in range(NCORES):
            eng = nc.sync
            eng.dma_start(
                out=h_rhs[h][:, core * 4:(core + 1) * 4, 0:NHID],
                in_=cc_h_out[h][core, :, :].rearrange("(l p) o -> p l o", p=128))

    stA.close()

    # =================== Stage B: layer-1 attention ========================
    stB = ExitStack()
    sb_ = stB.enter_context(tc.tile_pool(name="sb", bufs=1))
    psB_ctx = ExitStack()
    psB = psB_ctx.enter_context(tc.tile_pool(name="psB", bufs=1, space="PSUM"))

    GG = 8

    def attend_tiles(src_tile, src02_tile, sdst_ap_fn, sdst02_ap_fn, q_sink, goff):
        """Emit the 32 j-block elementwise chain; call q_sink(jb, q_slice)."""
        for jbg in range(NJB // GG):
            route_act = ((goff + jbg) % 2) == 0
            eL = sb_.tile([128, GG, R], bf16, tag="eL", bufs=3)
            if route_act:
                for j4 in range(GG):
                    jb = jbg * GG + j4
                    nc.scalar.activation(out=eL[:, j4, :], in_=src_tile,
                                         func=AF.Prelu, bias=sdst_ap_fn(jb),
                                         scale=1.0, alpha=ALPHA)
            else:
                t4 = sb_.tile([128, GG, R], bf16, tag="t4", bufs=2)
                e5 = sb_.tile([128, GG, R], bf16, tag="e5", bufs=2)
                for j4 in range(GG):
                    jb = jbg * GG + j4
                    nc.vector.tensor_scalar(out=t4[:, j4, :], in0=src_tile,
                                            scalar1=sdst_ap_fn(jb), scalar2=None,
                                            op0=OP.add)
                    nc.vector.tensor_scalar(out=e5[:, j4, :], in0=src02_tile,
                                            scalar1=sdst02_ap_fn(jb), scalar2=None,
                                            op0=OP.add)
                nc.vector.tensor_tensor(out=eL, in0=t4, in1=e5, op=OP.max)
            q = sb_.tile([128, GG, R], bf16, tag="q", bufs=3)
            nc.scalar.activation(out=q, in_=eL, func=AF.Exp)
            nc.vector.tensor_tensor(out=q, in0=q,
                                    in1=mT[:, jbg * GG:(jbg + 1) * GG, :], op=OP.mult)
            for j4 in range(GG):
                q_sink(jbg * GG + j4, q[:, j4, :])

    for h in range(NHEAD):
        ps_att = psB.tile([NHID + 1, R], f32, tag="att", bufs=2)

        def sink(jb, qs, ps_att=ps_att, h=h):
            nc.tensor.matmul(ps_att, lhsT=h_rhs[h][:, jb, :], rhs=qs,
                             start=(jb == 0), stop=(jb == NJB - 1))

        attend_tiles(src_bc[h], src02_bc[h],
                     lambda jb, h=h: sdst[:, h, jb:jb + 1],
                     lambda jb, h=h: sdst02[:, h, jb:jb + 1], sink, goff=h * (NJB // GG))

        # normalize + ELU -> x_catT
        dinv = sb_.tile([1, R], f32, tag="dinv", bufs=2)
        nc.vector.reciprocal(out=dinv, in_=ps_att[NHID:NHID + 1, :])
        ps_bc = psB.tile([64, R], f32, tag="bc", bufs=2)
        nc.tensor.matmul(ps_bc, lhsT=ones64, rhs=dinv, start=True, stop=True)
        att_sb = sb_.tile([64, R], f32, tag="attsb", bufs=2)
        nc.scalar.copy(out=att_sb, in_=ps_att[0:NHID, :])
        nc.vector.tensor_tensor(out=att_sb, in0=att_sb, in1=ps_bc, op=OP.mult)
        attn = att_sb
        neg = sb_.tile([64, R], f32, tag="neg", bufs=2)
        nc.vector.tensor_scalar(out=neg, in0=attn, scalar1=0.0, scalar2=None,
                                op0=OP.min)
        q2 = sb_.tile([64, R], f32, tag="q2", bufs=2)
        nc.scalar.activation(out=q2, in_=neg, func=AF.Exp)
        pos = sb_.tile([64, R], f32, tag="pos", bufs=2)
        nc.vector.tensor_scalar(out=pos, in0=attn, scalar1=0.0, scalar2=-1.0,
                                op0=OP.max, op1=OP.add)
        nc.vector.tensor_tensor(out=xcatT[h // 2][64 * (h % 2):64 * (h % 2) + 64, :],
                                in0=pos, in1=q2, op=OP.add)

    psB_ctx.close()

    # =================== Stage C: h_out = x_cat @ W_out, s2, gathers =======
    stC = ExitStack()
    sc = stC.enter_context(tc.tile_pool(name="sc", bufs=1))
    psC_ctx = ExitStack()
    psC = psC_ctx.enter_context(tc.tile_pool(name="psC", bufs=1, space="PSUM"))

    wo_sb = sc.tile([128, 5, NCLASS], bf16)
    nc.sync.dma_start(out=wo_sb[:, 0:4, :],
                      in_=wo[0:NHEAD * NHID, :].rearrange("(k p) c -> p k c", p=128))
    nc.sync.dma_start(out=wo_sb[0:1, 4, :], in_=wo[NHEAD * NHID:NHEAD * NHID + 1, :])
    ao_sb = sc.tile([32, 2], f32)
    nc.sync.dma_start(out=ao_sb, in_=ao)

    ps_ho = psC.tile([128, 4, NCLASS], f32)
    for ib in range(4):
        isl = slice(ib * 128, (ib + 1) * 128)
        for k in range(5):
            if k < 4:
                nc.tensor.matmul(ps_ho[:, ib, :], lhsT=xcatT[k][:, isl],
                                 rhs=wo_sb[:, k, :], start=(k == 0), stop=False)
            else:
                nc.tensor.matmul(ps_ho[:, ib, :], lhsT=ones_row[:, isl],
                                 rhs=wo_sb[0:1, 4, :], start=False, stop=True)
    h_out_sb = sc.tile([128, 4, NCLASS], f32)
    nc.scalar.copy(out=h_out_sb, in_=ps_ho)
    h_out_bf = sc.tile([128, 4, NCLASS], bf16)
    nc.vector.tensor_copy(out=h_out_bf, in_=h_out_sb)
    for ib in range(4):
        nc.sync.dma_start(out=cc_ho_in[ib * 128:(ib + 1) * 128, :],
                          in_=h_out_bf[:, ib, :])
    houtT = sc.tile([32, 4, 128], f32)
    for ib in range(4):
        ps_t2 = psC.tile([32, 128], f32, tag="tr2", bufs=2)
        nc.tensor.transpose(ps_t2, h_out_sb[:, ib, :], ident128)
        nc.scalar.copy(out=houtT[:, ib, :], in_=ps_t2)
    ps_s2 = psC.tile([2, R], f32)
    nc.tensor.matmul(ps_s2, lhsT=ao_sb, rhs=houtT.rearrange("p a b -> p (a b)"),
                     start=True, stop=True)
    s2_sb = sc.tile([2, R], f32)
    nc.vector.tensor_copy(out=s2_sb, in_=ps_s2)
    nc.sync.dma_start(out=cc_s2_in, in_=s2_sb)

    nc.gpsimd.collective_compute(AG, mybir.AluOpType.bypass, replica_groups=groups,
                                 ins=[cc_s2_in[:]], outs=[cc_s2_out[:]])
    nc.gpsimd.collective_compute(AG, mybir.AluOpType.bypass, replica_groups=groups,
                                 ins=[cc_ho_in[:]], outs=[cc_ho_out[:]])

    row2 = cc_s2_in[0:1, :]
    bc2 = bass.AP(tensor=row2.tensor, offset=row2.offset, ap=[[0, 128]] + row2.ap[1:])
    src2f = sc.tile([128, R], f32)
    nc.sync.dma_start(out=src2f, in_=bc2)
    nc.vector.tensor_copy(out=src2_bc, in_=src2f)
    nc.vector.tensor_scalar(out=src202_bc, in0=src2f, scalar1=ALPHA,
                            scalar2=None, op0=OP.mult)
    for core in range(NCORES):
        nc.sync.dma_start(
            out=s2dst[:, core * 4:(core + 1) * 4],
            in_=cc_s2_out[core, 1, :].rearrange("(l p) -> p l", p=128))
    nc.vector.tensor_scalar(out=s2dst02, in0=s2dst, scalar1=ALPHA,
                            scalar2=None, op0=OP.mult)
    for core in range(NCORES):
        nc.gpsimd.dma_start(
            out=h2_rhs[:, core * 4:(core + 1) * 4, 0:NCLASS],
            in_=cc_ho_out[core, :, :].rearrange("(l p) c -> p l c", p=128))

    psC_ctx.close()

    # =================== Stage D: layer-2 attention + log_softmax ==========
    stD = ExitStack()
    sd = stD.enter_context(tc.tile_pool(name="sd", bufs=1))
    psD = stD.enter_context(tc.tile_pool(name="psD", bufs=1, space="PSUM"))

    ps_o2T = psD.tile([NCLASS + 1, R], f32)

    def sink2(jb, qs):
        nc.tensor.matmul(ps_o2T, lhsT=h2_rhs[:, jb, :], rhs=qs,
                         start=(jb == 0), stop=(jb == NJB - 1))

    attend_tiles(src2_bc, src202_bc, lambda jb: s2dst[:, jb:jb + 1],
                 lambda jb: s2dst02[:, jb:jb + 1], sink2, goff=0)

    o2T_sb = sd.tile([NCLASS + 1, R], f32)
    nc.scalar.copy(out=o2T_sb, in_=ps_o2T)
    for ib in range(4):
        ps_row = psD.tile([128, NCLASS + 1], f32, tag="o2row", bufs=2)
        nc.tensor.transpose(ps_row, o2T_sb[:, ib * 128:(ib + 1) * 128], ident33)
        dinv2 = sd.tile([128, 1], f32, tag="dinv2", bufs=2)
        nc.vector.reciprocal(out=dinv2, in_=ps_row[:, NCLASS:NCLASS + 1])
        o2 = sd.tile([128, NCLASS], f32, tag="o2", bufs=2)
        nc.vector.tensor_scalar(out=o2, in0=ps_row[:, 0:NCLASS], scalar1=dinv2,
                                scalar2=None, op0=OP.mult)
        mx = sd.tile([128, 1], f32, tag="mx", bufs=2)
        nc.vector.tensor_reduce(out=mx, in_=o2, axis=mybir.AxisListType.X, op=OP.max)
        negmx = sd.tile([128, 1], f32, tag="negmx", bufs=2)
        nc.vector.tensor_scalar(out=negmx, in0=mx, scalar1=-1.0, scalar2=None,
                                op0=OP.mult)
        eo = sd.tile([128, NCLASS], f32, tag="eo", bufs=2)
        nc.scalar.activation(out=eo, in_=o2, func=AF.Exp, bias=negmx)
        se = sd.tile([128, 1], f32, tag="se", bufs=2)
        nc.vector.tensor_reduce(out=se, in_=eo, axis=mybir.AxisListType.X, op=OP.add)
        lse = sd.tile([128, 1], f32, tag="lse", bufs=2)
        nc.scalar.activation(out=lse, in_=se, func=AF.Ln)
        b2 = sd.tile([128, 1], f32, tag="b2", bufs=2)
        nc.vector.tensor_tensor(out=b2, in0=mx, in1=lse, op=OP.add)
        res = sd.tile([128, NCLASS], f32, tag="res", bufs=2)
        nc.vector.tensor_scalar(out=res, in0=o2, scalar1=b2, scalar2=None,
                                op0=OP.subtract)
        nc.sync.dma_start(out=out[ib * 128:(ib + 1) * 128, :], in_=res)

    stD.close()
    stC.close()
    stB.close()
    cst_ctx.close()


def _prep_inputs(x, adj, W_heads, b_heads, a_heads, W_out, b_out, a_out):
    """Host-side layout prep (slicing/transpose/dtype only)."""
    x = np.asarray(x, dtype=np.float32)
    adj = np.asarray(adj)
    W_heads = np.asarray(W_heads, dtype=np.float32)
    b_heads = np.asarray(b_heads, dtype=np.float32)
    a_heads = np.asarray(a_heads, dtype=np.float32)
    W_out = np.asarray(W_out, dtype=np.float32)
    b_out = np.asarray(b_out, dtype=np.float32)
    a_out = np.asarray(a_out, dtype=np.float32)

    wh = np.concatenate([W_heads, b_heads[:, None, :]], axis=1).astype(BF)
    aT = np.stack([a_heads[:, :NHID], a_heads[:, NHID:]], axis=2)  # [8, 64, 2]
    aT = np.ascontiguousarray(aT).astype(BF)
    wo = np.concatenate([W_out, b_out[None, :]], axis=0).astype(BF)  # [513, 32]
    ao = np.stack([a_out[:NCLASS], a_out[NCLASS:]], axis=1)  # [32, 2]
    ao = np.ascontiguousarray(ao)

    in_maps = []
    for c in range(NCORES):
        rs = slice(c * R, (c + 1) * R)
        xTc = np.concatenate([np.ascontiguousarray(x[rs].T),
                              np.ones((1, R), np.float32)], axis=0).astype(BF)
        adjTc = np.ascontiguousarray(adj[rs].T).astype(BF)
        in_maps.append({"xT": xTc, "wh": wh, "adjT": adjTc, "aT": aT,
                        "wo": wo, "ao": ao})
    return in_maps


def kernel(**inputs) -> np.ndarray:
    if "nc" not in _cached:
        _cached["nc"] = _build_program()
    nc = _cached["nc"]
    in_maps = _prep_inputs(**inputs)
    last_err = None
    for _attempt in range(3):
        try:
            res = run_bass_kernel_spmd(nc, in_maps, list(range(NCORES)))
            return np.concatenate([res.results[c]["out"] for c in range(NCORES)],
                                  axis=0)
        except Exception as e:  # transient device errors: retry
            last_err = e
            time.sleep(2)
    raise last_err



# revision 17
# speedup vs baseline: 1.1004x; 1.1004x over previous
"""GAT (2-layer graph attention network) on 8 Trainium2 NeuronCores.

Strategy: shard the node dim N=4096 across 8 cores (R=512 rows each); each
core computes its [512, 4096] slice of each attention matrix (row-softmax is
local); h is all-gathered.

Key algebraic trick (vs the previous version): the attention weights
factorize so that NO N^2 transcendental work is needed.

  e_ij = leakyrelu(s_i + t_j),  exp(e_ij) = max(exp(s_i+t_j), exp(a(s_i+t_j)))
       = exp(a*s_i) * max(w_i*v1_j, v2_j)
  with w = exp((1-a)s_src), v1 = exp(s_dst), v2 = exp(a*s_dst).
  The exp(a*s_i) row factor cancels in the softmax, so the masked softmax
  numerator is rho_ij = adj_ij * max(w_i*v1_j, v2_j) and the attended output
  is (rho @ [h|1]) rows 0..63 divided by row 64 (the denominator).

  w/v1/v2 are O(N)-sized vectors (exp'd once); per (head, j-block 128) tile
  the N^2 work is exactly two cheap ops:
     pass1: m = max(w_bc * v1_j, v2_j)   (tensor_scalar, 2 per-partition
            scalars, 4x DVE mode  -- or Relu(w*v1 - v2) on ACT)
     pass2: rho = m * mask               (tensor_tensor 2x -- or STT/GPSIMD)
  routed across DVE/ACT/GPSIMD to balance engine load ("D"/"A"/"G" routes).

Collectives are fused: one small s_dst gather + one fused h gather per layer
(the gathered h rows carry the lhsT ones-column inline so the post-gather
DMA is a single large contiguous-line transfer).
"""
import sys
import time

sys.path.insert(0, "/opt/trn_rl_repo")

import numpy as np
import ml_dtypes

import concourse.bass as bass
import concourse.bacc as bacc
import concourse.tile as tile
from concourse import mybir
from concourse.bass_utils import run_bass_kernel_spmd
from concourse.masks import make_identity

dt = mybir.dt
BF = ml_dtypes.bfloat16

N, NFEAT, NHID, NHEAD, NCLASS = 4096, 1024, 64, 8, 32
NCORES = 8
R = N // NCORES          # 512 rows per core
NJB = N // 128           # 32 j-blocks
KCH = NFEAT // 128       # 8 full K chunks for x@W (+1 bias row)
ALPHA = 0.2

# route pattern over global (head, group) slots: A = ACT relu + DVE STT,
# D = DVE tensor_scalar max + DVE tensor_tensor, G = DVE ts + GPSIMD tt.
PATTERN = ['A', 'D', 'A', 'G', 'A', 'A', 'D', 'G']
GG = 4                   # j-blocks per group

_cached = {}


def _build_program():
    nc = bacc.Bacc("TRN2", target_bir_lowering=False, debug=False,
                   enable_asserts=False, num_devices=NCORES)

    xT = nc.dram_tensor("xT", [NFEAT + 1, R], dt.bfloat16, kind="ExternalInput").ap()
    wh = nc.dram_tensor("wh", [NHEAD, NFEAT + 1, NHID], dt.bfloat16, kind="ExternalInput").ap()
    adjT = nc.dram_tensor("adjT", [N, R], dt.bfloat16, kind="ExternalInput").ap()
    aT = nc.dram_tensor("aT", [NHEAD, NHID, 2], dt.bfloat16, kind="ExternalInput").ap()
    wo = nc.dram_tensor("wo", [NHEAD * NHID + 1, NCLASS], dt.bfloat16, kind="ExternalInput").ap()
    ao = nc.dram_tensor("ao", [NCLASS, NCLASS + 1], dt.bfloat16, kind="ExternalInput").ap()
    out = nc.dram_tensor("out", [R, NCLASS], dt.float32, kind="ExternalOutput").ap()

    with tile.TileContext(nc, num_cores=NCORES) as tc:
        _emit(nc, tc, xT, wh, adjT, aT, wo, ao, out)
    nc.compile()
    return nc


def _emit(nc, tc, xT, wh, adjT, aT, wo, ao, out):
    from contextlib import ExitStack
    f32, bf16 = dt.float32, dt.bfloat16
    AF = mybir.ActivationFunctionType
    OP = mybir.AluOpType
    AG = "AllGather"
    groups = [list(range(NCORES))]

    cst_ctx = ExitStack()
    cst = cst_ctx.enter_context(tc.tile_pool(name="cst", bufs=1))
    dram = cst_ctx.enter_context(tc.tile_pool(name="dram", bufs=1, space="DRAM"))

    # ---- collective buffers ----
    cc_s_in = dram.tile([128, 4 * NHEAD], f32)
    cc_s_out = dram.tile([NCORES, 128, 4 * NHEAD], f32, addr_space="Shared")
    cc_h_in = dram.tile([R, NHEAD * 65], bf16)
    cc_h_out = dram.tile([NCORES, R, NHEAD * 65], bf16, addr_space="Shared")
    cc_s2_in = dram.tile([128, 4], f32)
    cc_s2_out = dram.tile([NCORES, 128, 4], f32, addr_space="Shared")
    cc_ho_in = dram.tile([R, NCLASS + 1], bf16)
    cc_ho_out = dram.tile([NCORES, R, NCLASS + 1], bf16, addr_space="Shared")

    # ---- persistent SBUF ----
    mT = cst.tile([128, NJB, R], bf16)              # raw 0/1 mask, transposed
    h_rhs = cst.tile([128, NJB, NHEAD * 65], bf16)  # gathered [h | 1] rows
    w_bc = cst.tile([128, NHEAD, R], bf16)
    v1 = cst.tile([128, NCORES, 4 * NHEAD], f32)    # [p, c, l*8+h]
    v2 = cst.tile([128, NCORES, 4 * NHEAD], f32)
    v2n = cst.tile([128, NCORES, 4 * NHEAD], f32)
    denA = cst.tile([97, R], f32)   # heads 0-3 at rows 0/32/64/96
    denB = cst.tile([97, R], f32)   # heads 4-7
    nc.vector.memset(denA, 1.0)
    nc.vector.memset(denB, 1.0)
    att_sb = cst.tile([128, 4, R], bf16)            # per-pair att rows
    xcatT = cst.tile([128, 4, R], bf16)
    h2_rhs = cst.tile([128, NJB, NCLASS + 1], bf16)
    w2_bc = cst.tile([128, R], bf16)
    v21 = cst.tile([128, NCORES, 4], f32)
    v22 = cst.tile([128, NCORES, 4], f32)
    v22n = cst.tile([128, NCORES, 4], f32)

    identB = cst.tile([128, 128], bf16)
    make_identity(nc, identB)
    ident8 = cst.tile([8, 8], f32)
    make_identity(nc, ident8)
    ident33 = cst.tile([NCLASS + 1, NCLASS + 1], f32)
    make_identity(nc, ident33)
    ones_col = cst.tile([1, 128], f32)
    nc.vector.memset(ones_col, 1.0)
    ones11 = cst.tile([1, 1], f32)
    nc.vector.memset(ones11, 1.0)
    ones_row = cst.tile([1, R], bf16)
    nc.vector.memset(ones_row, 1.0)
    sel97 = cst.tile([97, 128], f32)                # pair denominator bcast
    nc.vector.memset(sel97, 0.0)
    nc.vector.memset(sel97[0:1, 0:64], 1.0)
    nc.vector.memset(sel97[32:33, 64:128], 1.0)
    nc.vector.memset(sel97[64:65, 0:64], 1.0)
    nc.vector.memset(sel97[96:97, 64:128], 1.0)

    # =================== Stage A =====================================
    stA = ExitStack()
    sa = stA.enter_context(tc.tile_pool(name="sa", bufs=1))
    psA = stA.enter_context(tc.tile_pool(name="psA", bufs=1, space="PSUM"))

    aT_sb = sa.tile([64, NHEAD, 2], bf16)
    nc.sync.dma_start(out=aT_sb, in_=aT.rearrange("h o k -> o h k"))
    xT_sb = sa.tile([128, KCH + 1, R], bf16)
    nc.sync.dma_start(out=xT_sb[:, 0:KCH, :],
                      in_=xT[0:NFEAT, :].rearrange("(k p) i -> p k i", p=128))
    nc.sync.dma_start(out=xT_sb[0:1, KCH, :], in_=xT[NFEAT:NFEAT + 1, :])
    wh_sb = sa.tile([128, 4, KCH + 1, 2, NHID], bf16)   # [p, pair, k, h%2, o]
    for h in range(NHEAD):
        nc.sync.dma_start(out=wh_sb[:, h // 2, 0:KCH, h % 2, :],
                          in_=wh[h, 0:NFEAT, :].rearrange("(k p) o -> p k o", p=128))
        nc.sync.dma_start(out=wh_sb[0:1, h // 2, KCH, h % 2, :],
                          in_=wh[h, NFEAT:NFEAT + 1, :])
    # big mask load on the scalar HWDGE ring so it doesn't delay the
    # collective-feeding DMAs on the sync ring
    nc.scalar.dma_start(out=mT, in_=adjT.rearrange("(jb p) i -> p jb i", p=128))
    wo_sb = cst.tile([128, 5, NCLASS], bf16)
    nc.scalar.dma_start(out=wo_sb[:, 0:4, :],
                        in_=wo[0:NHEAD * NHID, :].rearrange("(k p) c -> p k c", p=128))
    nc.scalar.dma_start(out=wo_sb[0:1, 4, :],
                        in_=wo[NHEAD * NHID:NHEAD * NHID + 1, :])
    ao_sb = cst.tile([NCLASS, NCLASS + 1], bf16)
    nc.scalar.dma_start(out=ao_sb, in_=ao)

    # zero-padded block-diag attention-vector lhsT: per pair pr, col 2pr
    # (rows 0-63) = head 2pr's vector, col 2pr+1 (rows 64-127) = head 2pr+1's
    a2s = sa.tile([128, 4, NHEAD], bf16)
    a2d = sa.tile([128, 4, NHEAD], bf16)
    nc.vector.memset(a2s, 0.0)
    nc.vector.memset(a2d, 0.0)
    for pr in range(4):
        h0, h1 = 2 * pr, 2 * pr + 1
        nc.vector.tensor_copy(out=a2s[0:64, pr, h0:h0 + 1], in_=aT_sb[:, h0, 0:1])
        nc.vector.tensor_copy(out=a2s[64:128, pr, h1:h1 + 1], in_=aT_sb[:, h1, 0:1])
        nc.vector.tensor_copy(out=a2d[0:64, pr, h0:h0 + 1], in_=aT_sb[:, h0, 1:2])
        nc.vector.tensor_copy(out=a2d[64:128, pr, h1:h1 + 1], in_=aT_sb[:, h1, 1:2])

    s_src = sa.tile([NHEAD, R], f32)
    s_dst = sa.tile([NHEAD, R], f32)
    s_dram = dram.tile([NHEAD, R], f32)
    hT_sb = sa.tile([128, 4, R], bf16)
    h_row = sa.tile([128, 4, NHEAD, 65], bf16)
    nc.vector.memset(h_row[:, :, :, 64:65], 1.0)

    ps_ss = psA.tile([NHEAD, R], f32, tag="ss")
    ps_sd = psA.tile([NHEAD, R], f32, tag="sd")
    for pr in range(4):
        ps_hT = psA.tile([128, R], f32, tag="hT", bufs=2)
        for k in range(KCH + 1):
            kp = 128 if k < KCH else 1
            nc.tensor.matmul(ps_hT,
                             lhsT=wh_sb[0:kp, pr, k, :, :].rearrange(
                                 "p a b -> p (a b)"),
                             rhs=xT_sb[0:kp, k, :],
                             start=(k == 0), stop=(k == KCH))
        nc.scalar.copy(out=hT_sb[:, pr, :], in_=ps_hT)
        nc.tensor.matmul(ps_ss, lhsT=a2s[:, pr, :], rhs=hT_sb[:, pr, :],
                         start=(pr == 0), stop=(pr == 3))
        nc.tensor.matmul(ps_sd, lhsT=a2d[:, pr, :], rhs=hT_sb[:, pr, :],
                         start=(pr == 0), stop=(pr == 3))
        for l in range(4):
            ps_tr = psA.tile([128, 128], bf16, tag="tr", bufs=2)
            nc.tensor.transpose(ps_tr, hT_sb[:, pr, l * 128:(l + 1) * 128], identB)
            nc.scalar.copy(out=h_row[:, l, 2 * pr:2 * pr + 2, 0:64],
                           in_=ps_tr.rearrange("p (a b) -> p a b", a=2))

    nc.scalar.copy(out=s_src, in_=ps_ss)
    nc.scalar.copy(out=s_dst, in_=ps_sd)
    # local transposed s_dst -> collective
    ps_st = psA.tile([128, 4, NHEAD], f32, tag="st")
    for l in range(4):
        nc.tensor.transpose(ps_st[:, l, :], s_dst[:, l * 128:(l + 1) * 128], ident8)
    sT_loc = sa.tile([128, 4, NHEAD], f32)
    nc.vector.tensor_copy(out=sT_loc, in_=ps_st)
    nc.sync.dma_start(out=cc_s_in, in_=sT_loc.rearrange("p l h -> p (l h)"))
    nc.gpsimd.collective_compute(AG, OP.bypass, replica_groups=groups,
                                 ins=[cc_s_in[:]], outs=[cc_s_out[:]])
    nc.sync.dma_start(out=cc_h_in.rearrange("(l p) x -> p l x", p=128),
                      in_=h_row.rearrange("p l h o -> p l (h o)"))
    nc.gpsimd.collective_compute(AG, OP.bypass, replica_groups=groups,
                                 ins=[cc_h_in[:]], outs=[cc_h_out[:]])

    # w broadcast (gather-independent): w_bc[h] = exp(0.8 * s_src[h]) bcast
    # via DRAM round-trip + partition-stride-0 broadcast DMA
    nc.sync.dma_start(out=s_dram, in_=s_src)
    s_bc = sa.tile([128, NHEAD, R], f32)
    sd_ap = s_dram[:]
    bc_ap = bass.AP(tensor=sd_ap.tensor, offset=sd_ap.offset,
                    ap=[[0, 128]] + list(sd_ap.ap))
    nc.sync.dma_start(out=s_bc, in_=bc_ap)
    for h in range(NHEAD):
        nc.scalar.activation(out=w_bc[:, h, :], in_=s_bc[:, h, :], func=AF.Exp,
                             scale=1.0 - ALPHA)

    # gather consumers
    sdraw = sa.tile([128, NCORES, 4 * NHEAD], f32)
    nc.sync.dma_start(out=sdraw, in_=cc_s_out.rearrange("c p x -> p c x"))
    sdraw2d = sdraw.rearrange("p a b -> p (a b)")
    nc.scalar.activation(out=v1.rearrange("p a b -> p (a b)"), in_=sdraw2d,
                         func=AF.Exp)
    nc.scalar.activation(out=v2.rearrange("p a b -> p (a b)"), in_=sdraw2d,
                         func=AF.Exp, scale=ALPHA)
    nc.vector.tensor_scalar(out=v2n.rearrange("p a b -> p (a b)"),
                            in0=v2.rearrange("p a b -> p (a b)"),
                            scalar1=-1.0, scalar2=None, op0=OP.mult)
    nc.sync.dma_start(out=h_rhs,
                      in_=cc_h_out.rearrange("c (l p) x -> p (c l) x", p=128))

    stA.close()

    # =================== attention helper ============================
    stB = ExitStack()
    sb_ = stB.enter_context(tc.tile_pool(name="sb", bufs=1))
    psB_ctx = ExitStack()
    psB = psB_ctx.enter_context(tc.tile_pool(name="psB", bufs=1, space="PSUM"))

    slot = [0]

    def attend(wbc_ap, v1_fn, v2_fn, v2n_fn, sink):
        """32 j-blocks of rho = mask * max(w*v1, v2); sink(jb, rho_slice)."""
        for g in range(NJB // GG):
            jb0 = g * GG
            route = PATTERN[slot[0] % len(PATTERN)]
            slot[0] += 1
            rho = sb_.tile([128, GG, R], bf16, tag="rho", bufs=4)
            if route == 'A':
                r = sb_.tile([128, GG, R], bf16, tag="r", bufs=2)
                for j in range(GG):
                    jb = jb0 + j
                    nc.scalar.activation(out=r[:, j, :], in_=wbc_ap, func=AF.Relu,
                                         scale=v1_fn(jb), bias=v2n_fn(jb))
                    nc.vector.scalar_tensor_tensor(out=rho[:, j, :], in0=r[:, j, :],
                                                   scalar=v2_fn(jb),
                                                   in1=mT[:, jb, :],
                                                   op0=OP.add, op1=OP.mult)
            else:
                m = sb_.tile([128, GG, R], bf16, tag="m", bufs=2)
                for j in range(GG):
                    jb = jb0 + j
                    nc.vector.tensor_scalar(out=m[:, j, :], in0=wbc_ap,
                                            scalar1=v1_fn(jb), scalar2=v2_fn(jb),
                                            op0=OP.mult, op1=OP.max)
                eng = nc.vector if route == 'D' else nc.gpsimd
                eng.tensor_tensor(out=rho, in0=m,
                                  in1=mT[:, jb0:jb0 + GG, :], op=OP.mult)
            for j in range(GG):
                sink(jb0 + j, rho[:, j, :])

    # =================== Stage B: layer-1 attention ==================
    for h in range(NHEAD):
        ps_att = psB.tile([65, R], f32, tag="att", bufs=2)

        def sink(jb, q, ps_att=ps_att, h=h):
            nc.tensor.matmul(ps_att, lhsT=h_rhs[:, jb, h * 65:(h + 1) * 65],
                             rhs=q, start=(jb == 0), stop=(jb == NJB - 1))

        attend(w_bc[:, h, :],
               lambda jb, h=h: v1[:, jb // 4, (jb % 4) * NHEAD + h:(jb % 4) * NHEAD + h + 1],
               lambda jb, h=h: v2[:, jb // 4, (jb % 4) * NHEAD + h:(jb % 4) * NHEAD + h + 1],
               lambda jb, h=h: v2n[:, jb // 4, (jb % 4) * NHEAD + h:(jb % 4) * NHEAD + h + 1],
               sink)
        den_t = denA if h < 4 else denB
        hh = h % 4
        nc.scalar.copy(out=den_t[32 * hh:32 * hh + 1, :], in_=ps_att[64:65, :])
        nc.scalar.copy(out=att_sb[64 * (h % 2):64 * (h % 2) + 64, h // 2, :],
                       in_=ps_att[0:64, :])

    recA = sb_.tile([97, R], f32, tag="recA")
    recB = sb_.tile([97, R], f32, tag="recB")
    nc.vector.reciprocal(out=recA, in_=denA)
    nc.vector.reciprocal(out=recB, in_=denB)
    for pr in range(4):
        rec_t = recA if pr < 2 else recB
        rbase = 64 * (pr % 2)
        ps_db = psB.tile([128, R], f32, tag="db", bufs=2)
        nc.tensor.matmul(ps_db, lhsT=sel97[rbase:rbase + 33, :],
                         rhs=rec_t[rbase:rbase + 33, :],
                         start=True, stop=True)
        dbc = sb_.tile([128, R], bf16, tag="dbc", bufs=2)
        nc.scalar.copy(out=dbc, in_=ps_db)
        u = sb_.tile([128, R], bf16, tag="u", bufs=2)
        nc.vector.tensor_tensor(out=u, in0=att_sb[:, pr, :], in1=dbc, op=OP.mult)
        neg = sb_.tile([128, R], bf16, tag="neg", bufs=2)
        nc.vector.tensor_scalar(out=neg, in0=u, scalar1=0.0, scalar2=None,
                                op0=OP.min)
        eneg = sb_.tile([128, R], bf16, tag="eneg", bufs=2)
        nc.scalar.activation(out=eneg, in_=neg, func=AF.Exp)
        pos = sb_.tile([128, R], bf16, tag="pos", bufs=2)
        nc.vector.tensor_scalar(out=pos, in0=u, scalar1=0.0, scalar2=-1.0,
                                op0=OP.max, op1=OP.add)
        nc.vector.tensor_tensor(out=xcatT[:, pr, :], in0=pos, in1=eneg, op=OP.add)

    # =================== Stage C: h_out, layer-2 prep ================
    psB_ctx.close()
    stC = ExitStack()
    psC = stC.enter_context(tc.tile_pool(name="psC", bufs=1, space="PSUM"))
    ps_ho = psC.tile([128, 4, NCLASS], f32, tag="ho")
    for ib in range(4):
        isl = slice(ib * 128, (ib + 1) * 128)
        for k in range(4):
            nc.tensor.matmul(ps_ho[:, ib, :], lhsT=xcatT[:, k, isl],
                             rhs=wo_sb[:, k, :], start=(k == 0), stop=False)
        nc.tensor.matmul(ps_ho[:, ib, :], lhsT=ones_row[:, isl],
                         rhs=wo_sb[0:1, 4, :], start=False, stop=True)
    ho_row = sb_.tile([128, 4, NCLASS + 1], bf16, tag="horow")
    nc.vector.memset(ho_row[:, :, NCLASS:NCLASS + 1], 1.0)
    nc.scalar.copy(out=ho_row[:, :, 0:NCLASS], in_=ps_ho)
    ps_hoT = psC.tile([NCLASS, 4, 128], bf16, tag="hoT")
    for l in range(4):
        nc.tensor.transpose(ps_hoT[:, l, :], ho_row[:, l, 0:NCLASS], identB)
    hoT_sb = sb_.tile([NCLASS, 4, 128], bf16, tag="hoTs")
    nc.vector.tensor_copy(out=hoT_sb, in_=ps_hoT)
    ps_s2 = psC.tile([NCLASS + 1, R], f32, tag="s2")
    nc.tensor.matmul(ps_s2, lhsT=ao_sb, rhs=hoT_sb.rearrange("p a b -> p (a b)"),
                     start=True, stop=True)
    s2s_sb = sb_.tile([1, R], f32, tag="s2s")      # src row (ao col 0)
    nc.vector.tensor_copy(out=s2s_sb, in_=ps_s2[0:1, :])
    s2d_sb = sb_.tile([1, R], f32, tag="s2d")      # dst row (ao col 32)
    nc.vector.tensor_copy(out=s2d_sb, in_=ps_s2[32:33, :])
    ps_s2T = psC.tile([128, 4], f32, tag="s2T")
    for l in range(4):
        nc.tensor.matmul(ps_s2T[:, l:l + 1], lhsT=s2d_sb[0:1, l * 128:(l + 1) * 128],
                         rhs=ones11, start=True, stop=True)
    s2T_loc = sb_.tile([128, 4], f32, tag="s2Tl")
    nc.vector.tensor_copy(out=s2T_loc, in_=ps_s2T)
    nc.sync.dma_start(out=cc_s2_in, in_=s2T_loc)
    nc.gpsimd.collective_compute(AG, OP.bypass, replica_groups=groups,
                                 ins=[cc_s2_in[:]], outs=[cc_s2_out[:]])
    nc.sync.dma_start(out=cc_ho_in.rearrange("(l p) x -> p l x", p=128),
                      in_=ho_row)
    nc.gpsimd.collective_compute(AG, OP.bypass, replica_groups=groups,
                                 ins=[cc_ho_in[:]], outs=[cc_ho_out[:]])

    ps_w2 = psC.tile([128, R], f32, tag="w2")
    nc.tensor.matmul(ps_w2, lhsT=ones_col, rhs=s2s_sb, start=True, stop=True)
    nc.scalar.activation(out=w2_bc, in_=ps_w2, func=AF.Exp, scale=1.0 - ALPHA)

    sdraw2 = sb_.tile([128, NCORES, 4], f32, tag="sd2")
    nc.sync.dma_start(out=sdraw2, in_=cc_s2_out.rearrange("c p l -> p c l"))
    sdraw2_2d = sdraw2.rearrange("p a b -> p (a b)")
    nc.scalar.activation(out=v21.rearrange("p a b -> p (a b)"), in_=sdraw2_2d,
                         func=AF.Exp)
    nc.scalar.activation(out=v22.rearrange("p a b -> p (a b)"), in_=sdraw2_2d,
                         func=AF.Exp, scale=ALPHA)
    nc.vector.tensor_scalar(out=v22n.rearrange("p a b -> p (a b)"),
                            in0=v22.rearrange("p a b -> p (a b)"),
                            scalar1=-1.0, scalar2=None, op0=OP.mult)
    nc.sync.dma_start(out=h2_rhs,
                      in_=cc_ho_out.rearrange("c (l p) x -> p (c l) x", p=128))

    # =================== Stage D: layer-2 attention + log_softmax ====
    stC.close()
    stD = ExitStack()
    psD = stD.enter_context(tc.tile_pool(name="psD", bufs=1, space="PSUM"))
    ps_o2 = psD.tile([NCLASS + 1, R], f32, tag="o2acc")

    def sink2(jb, q):
        nc.tensor.matmul(ps_o2, lhsT=h2_rhs[:, jb, :], rhs=q,
                         start=(jb == 0), stop=(jb == NJB - 1))

    attend(w2_bc,
           lambda jb: v21[:, jb // 4, jb % 4:jb % 4 + 1],
           lambda jb: v22[:, jb // 4, jb % 4:jb % 4 + 1],
           lambda jb: v22n[:, jb // 4, jb % 4:jb % 4 + 1],
           sink2)

    o2T_sb = sb_.tile([NCLASS + 1, R], f32, tag="o2T")
    nc.scalar.copy(out=o2T_sb, in_=ps_o2)
    for ib in range(4):
        ps_row = psD.tile([128, NCLASS + 1], f32, tag="o2row", bufs=2)
        nc.tensor.transpose(ps_row, o2T_sb[:, ib * 128:(ib + 1) * 128], ident33)
        dinv2 = sb_.tile([128, 1], f32, tag="dinv2", bufs=2)
        nc.vector.reciprocal(out=dinv2, in_=ps_row[:, NCLASS:NCLASS + 1])
        o2 = sb_.tile([128, NCLASS], f32, tag="o2", bufs=2)
        nc.vector.tensor_scalar(out=o2, in0=ps_row[:, 0:NCLASS], scalar1=dinv2,
                                scalar2=None, op0=OP.mult)
        mx = sb_.tile([128, 1], f32, tag="mx", bufs=2)
        nc.vector.tensor_reduce(out=mx, in_=o2, axis=mybir.AxisListType.X, op=OP.max)
        negmx = sb_.tile([128, 1], f32, tag="negmx", bufs=2)
        nc.vector.tensor_scalar(out=negmx, in0=mx, scalar1=-1.0, scalar2=None,
                                op0=OP.mult)
        eo = sb_.tile([128, NCLASS], f32, tag="eo", bufs=2)
        nc.scalar.activation(out=eo, in_=o2, func=AF.Exp, bias=negmx)
        se = sb_.tile([128, 1], f32, tag="se", bufs=2)
        nc.vector.tensor_reduce(out=se, in_=eo, axis=mybir.AxisListType.X, op=OP.add)
        lse = sb_.tile([128, 1], f32, tag="lse", bufs=2)
        nc.scalar.activation(out=lse, in_=se, func=AF.Ln)
        b2 = sb_.tile([128, 1], f32, tag="b2", bufs=2)
        nc.vector.tensor_tensor(out=b2, in0=mx, in1=lse, op=OP.add)
        res = sb_.tile([128, NCLASS], f32, tag="res", bufs=2)
        nc.vector.tensor_scalar(out=res, in0=o2, scalar1=b2, scalar2=None,
                                op0=OP.subtract)
        nc.sync.dma_start(out=out[ib * 128:(ib + 1) * 128, :], in_=res)

    stD.close()
    stB.close()
    cst_ctx.close()


def _prep_inputs(x, adj, W_heads, b_heads, a_heads, W_out, b_out, a_out):
    """Host-side layout prep (slicing/transpose/dtype only)."""
    x = np.asarray(x, dtype=np.float32)
    adj = np.asarray(adj)
    W_heads = np.asarray(W_heads, dtype=np.float32)
    b_heads = np.asarray(b_heads, dtype=np.float32)
    a_heads = np.asarray(a_heads, dtype=np.float32)
    W_out = np.asarray(W_out, dtype=np.float32)
    b_out = np.asarray(b_out, dtype=np.float32)
    a_out = np.asarray(a_out, dtype=np.float32)

    wh = np.concatenate([W_heads, b_heads[:, None, :]], axis=1).astype(BF)
    aT = np.stack([a_heads[:, :NHID], a_heads[:, NHID:]], axis=2)  # [8, 64, 2]
    aT = np.ascontiguousarray(aT).astype(BF)
    wo = np.concatenate([W_out, b_out[None, :]], axis=0).astype(BF)  # [513, 32]
    ao = np.zeros((NCLASS, NCLASS + 1), np.float32)   # col 0 = src, col 32 = dst
    ao[:, 0] = a_out[:NCLASS]
    ao[:, NCLASS] = a_out[NCLASS:]
    ao = ao.astype(BF)

    in_maps = []
    for c in range(NCORES):
        rs = slice(c * R, (c + 1) * R)
        xTc = np.concatenate([np.ascontiguousarray(x[rs].T),
                              np.ones((1, R), np.float32)], axis=0).astype(BF)
        adjTc = np.ascontiguousarray(adj[rs].T).astype(BF)
        in_maps.append({"xT": xTc, "wh": wh, "adjT": adjTc, "aT": aT,
                        "wo": wo, "ao": ao})
    return in_maps


def kernel(**inputs) -> np.ndarray:
    if "nc" not in _cached:
        _cached["nc"] = _build_program()
    nc = _cached["nc"]
    in_maps = _prep_inputs(**inputs)
    last_err = None
    for _attempt in range(3):
        try:
            res = run_bass_kernel_spmd(nc, in_maps, list(range(NCORES)))
            return np.concatenate([res.results[c]["out"] for c in range(NCORES)],
                                  axis=0)
        except Exception as e:  # transient device errors: retry
            last_err = e
            time.sleep(2)
    raise last_err
